# revision 30
# baseline (speedup 1.0000x reference)
"""Trainium2 Bass kernel for nn_SinkhornLayer: 10 log-domain Sinkhorn iterations
on 64 independent [1024,1024] fp32 matrices, batch-sharded over 8 NeuronCores.

Algorithm (multiplicative domain, mathematically equal to log-domain reference):
    K  = exp(10*(M - rowmax(M)))      rowmax for overflow safety
    u1 = 1 / rowsum(K)                (rowsum fused into the exp pass on ScalarE)
    for t = 1..10:
        if t > 1:  u = 1 / (K v)      row-direction matvec
        v = 1 / (K^T u)               col-direction matvec
    out = diag(u) K diag(v)

K and K^T are held in SBUF in bf16 (numpy simulation of the bf16 pipeline vs
the fp32 reference gives ~3.6e-3 max rel err, well under the 2e-2 gate).

Matvec form: the 128x128 K tiles are the stationary (weight) operand and the
vector chunk [128,1] is the moving operand, so every half-iteration is 64
LDWEIGHTS+MATMUL pairs accumulating into a [128,8] PSUM column vector -- the
u/v iterates stay in column layout the whole time and the only non-PE op per
half-iteration is one tiny reciprocal. bf16 weights get the fast-weight-load
path (2 cols/cycle).
"""
import numpy as np
from contextlib import ExitStack

import concourse.bacc as bacc
import concourse.bass as bass
import concourse.tile as tile
from concourse import mybir
from concourse.bass_utils import run_bass_kernel_spmd
from concourse.masks import make_identity

F32 = mybir.dt.float32
BF16 = mybir.dt.bfloat16
AF = mybir.ActivationFunctionType
ALU = mybir.AluOpType

P = 128          # SBUF partitions
N = 1024         # matrix dim
B = 64           # batch
NCORES = 8
BPC = B // NCORES
TPM = N // P     # 8 row/col tiles per matrix
ITERS = 10
INV_EPS = 10.0


COLTILE = False


# 2 concurrent PE column groups, each streaming its own j-slice of the matrix
# on its own XBUS (xdata[0]/xdata[1]; 4 groups hits the quadrant-3 HW bug and
# 3 groups proved unstable under For_i).
CT_SPLITS = ((0, 512), (512, 1024))
# (psum row, free offset) holding column chunk tj of the matvec result
CT_CHUNK = ((0, 0), (0, 128), (0, 256), (0, 384),
            (32, 0), (32, 128), (32, 256), (32, 384))


def _half_iter_ct(tc, pools, mat, w, ones_col, fp32_out=False):
    """Column-tiled streaming half-iteration: bf16 [P, TPM] = 1/(mat^T w).
    mat: [P, TPM, N] bf16, streamed as the moving operand through 3 concurrent
    PE column groups; w: [P, TPM] bf16 column vector (the stationary operand,
    M=1). Result rows {0,32,64} of PSUM are transposed back to column layout.
    The MM block is a scheduling-atomic critical section: a full-array PE op
    (e.g. a transpose) interleaved between col-tiled MMs with live concurrent
    group streams wedges the PE."""
    nc = tc.nc
    psmv, pscol, sbmv, sbvec = pools
    mv = psmv.tile([P, 512], F32, tag="mv")
    with tc.tile_critical():
        for ti in range(TPM):
            for g, (j0, j1) in enumerate(CT_SPLITS):
                nc.tensor.matmul(
                    mv[32 * g:32 * g + 1, 0:j1 - j0],
                    w[:, ti:ti + 1],
                    mat[:, ti, j0:j1],
                    start=(ti == 0), stop=(ti == TPM - 1),
                    tile_position=(0, 32 * g), skip_group_check=True,
                )
    mv_sb = sbmv.tile([P, 512], F32, tag="mv_sb")
    nc.vector.tensor_copy(mv_sb, mv)
    sc = pscol.tile([P, TPM], F32, tag="sc")
    for tj, (row, off) in enumerate(CT_CHUNK):
        nc.tensor.transpose(
            sc[:, tj:tj + 1],
            mv_sb[row:row + 1, off:off + P],
            ones_col[row:row + 1, 0:1],
        )
    rb = sbvec.tile([P, TPM], BF16, tag="uvb")
    with nc.allow_low_precision("sinkhorn u/v iterates are bf16 by design"):
        nc.vector.reciprocal(rb, sc)
    if fp32_out:
        rf = sbvec.tile([P, TPM], F32, tag="uvf")
        nc.vector.reciprocal(rf, sc)
        return rb, rf
    return rb, None


def _half_iter(nc, psmv, sbvec, mat, w, fp32_out=False):
    """One half-iteration: returns bf16 [P, TPM] = 1/(mat^T w) in column layout.
    mat: [P, TPM, N] bf16 tiles (contraction over partitions);
    w:   [P, TPM] bf16 column-layout vector.
    If fp32_out, also returns an fp32 copy of the reciprocal."""
    ps = psmv.tile([P, TPM], F32, tag="mv")
    for ob in range(TPM):
        for ci in range(TPM):
            nc.tensor.matmul(
                ps[:, ob:ob + 1],
                mat[:, ci, ob * P:(ob + 1) * P],
                w[:, ci:ci + 1],
                start=(ci == 0), stop=(ci == TPM - 1),
            )
    rb = sbvec.tile([P, TPM], BF16, tag="uvb")
    with nc.allow_low_precision("sinkhorn u/v iterates are bf16 by design"):
        nc.vector.reciprocal(rb, ps)
    if fp32_out:
        rf = sbvec.tile([P, TPM], F32, tag="uvf")
        nc.vector.reciprocal(rf, ps)
        return rb, rf
    return rb, None


def sinkhorn_kernel(ctx, tc, out_ap, m_ap, reps=1, alias_io=False):
    nc = tc.nc
    const = ctx.enter_context(tc.tile_pool(name="const", bufs=1))
    ident = const.tile([P, P], BF16)
    make_identity(nc, ident[:])
    identf = const.tile([P, P], F32)
    make_identity(nc, identf[:])
    ones_row = const.tile([1, P], F32)
    nc.vector.memset(ones_row, 1.0)
    ones_col = const.tile([P, 1], F32)
    nc.vector.memset(ones_col, 1.0)

    kpool = ctx.enter_context(tc.tile_pool(name="kmat", bufs=2))
    ktpool = ctx.enter_context(tc.tile_pool(name="ktmat", bufs=2))
    ppool = ctx.enter_context(tc.tile_pool(name="p0", bufs=3))
    epool = ctx.enter_context(tc.tile_pool(name="eout", bufs=3))
    sbvec = ctx.enter_context(tc.tile_pool(name="sbvec", bufs=4))
    sbrow = ctx.enter_context(tc.tile_pool(name="sbrow", bufs=2))

    psmv = ctx.enter_context(tc.tile_pool(name="psmv", bufs=2, space="PSUM"))
    pscol = ctx.enter_context(tc.tile_pool(name="pscol", bufs=2, space="PSUM"))
    pstr = ctx.enter_context(tc.tile_pool(name="pstr", bufs=2, space="PSUM"))
    psbig = ctx.enter_context(tc.tile_pool(name="psbig", bufs=2, space="PSUM"))
    sbmv = ctx.enter_context(tc.tile_pool(name="sbmv", bufs=2))
    mv_pools = (psmv, pscol, sbmv, sbvec)

    for rep in range(reps):
      for b in range(BPC):
        bi = 0 if alias_io else b
        # ---- phase 1: load, rowmax, K = exp(10*(M - rowmax)) bf16, rowsum ----
        ktb = kpool.tile([P, TPM, N], BF16, tag="ktb")
        negmx = sbvec.tile([P, TPM], F32, tag="negmx")
        rowsum = sbvec.tile([P, TPM], F32, tag="rowsum")
        for ti in range(TPM):
            p0 = ppool.tile([P, N], F32, tag="p0")
            nc.sync.dma_start(out=p0, in_=m_ap[bi, ti * P:(ti + 1) * P, :])
            nc.vector.reduce_max(negmx[:, ti:ti + 1], p0,
                                 axis=mybir.AxisListType.X, negate=True)
            nc.vector.tensor_scalar_mul(negmx[:, ti:ti + 1], negmx[:, ti:ti + 1],
                                        INV_EPS)
            nc.scalar.activation(out=ktb[:, ti, :], in_=p0, func=AF.Exp,
                                 bias=negmx[:, ti:ti + 1], scale=INV_EPS,
                                 accum_out=rowsum[:, ti:ti + 1])
        ub = sbvec.tile([P, TPM], BF16, tag="uvb")
        with nc.allow_low_precision("sinkhorn u/v iterates are bf16 by design"):
            nc.vector.reciprocal(ub, rowsum)

        # ---- phase 2: K^T (bf16) via 64 PE block transposes, 8 PSUM drains ----
        kttb = ktpool.tile([P, TPM, N], BF16, tag="kttb")
        for tj in range(TPM):
            pt = pstr.tile([P, N], BF16, tag="pt")
            for ti in range(TPM):
                nc.tensor.transpose(pt[:, ti * P:(ti + 1) * P],
                                    ktb[:, ti, tj * P:(tj + 1) * P], ident)
            nc.vector.tensor_copy(kttb[:, tj, :], pt)

        # ---- phase 3: Sinkhorn iterations (bf16 weights, column vectors) ----
        u, v = ub, None
        uf = vf = None
        for t in range(ITERS):
            last = (t == ITERS - 1)
            if COLTILE:
                if t > 0:
                    u, uf = _half_iter_ct(tc, mv_pools, kttb, v, ones_col,
                                          fp32_out=last)
                v, vf = _half_iter_ct(tc, mv_pools, ktb, u, ones_col,
                                      fp32_out=last)
            else:
                if t > 0:
                    u, uf = _half_iter(nc, psmv, sbvec, kttb, v, fp32_out=last)
                v, vf = _half_iter(nc, psmv, sbvec, ktb, u, fp32_out=last)

        # ---- phase 4: out = diag(u) K diag(v) ----
        # v (fp32, col layout) -> row [1, N] via 8 PE transposes, broadcast to
        # [P, N] with a ones-matmul, then one fused DVE op per row chunk.
        vrow_sb = sbrow.tile([1, N], F32, tag="vrow")
        for h in range(2):
            vr_ps = pscol.tile([1, N // 2], F32, tag="sc")
            for k in range(4):
                tj = 4 * h + k
                nc.tensor.transpose(vr_ps[0:1, k * P:(k + 1) * P],
                                    vf[:, tj:tj + 1], identf)
            nc.any.tensor_copy(vrow_sb[0:1, h * (N // 2):(h + 1) * (N // 2)], vr_ps)
        vb = []
        for h in range(2):
            vbh = psbig.tile([P, N // 2], F32, tag="psb")
            nc.tensor.matmul(vbh, ones_row,
                             vrow_sb[0:1, h * (N // 2):(h + 1) * (N // 2)],
                             start=True, stop=True)
            vb.append(vbh)
        for ti in range(TPM):
            e = epool.tile([P, N], F32, tag="e")
            for h in range(2):
                nc.vector.scalar_tensor_tensor(
                    out=e[:, h * (N // 2):(h + 1) * (N // 2)],
                    in0=ktb[:, ti, h * (N // 2):(h + 1) * (N // 2)],
                    scalar=uf[:, ti:ti + 1],
                    in1=vb[h],
                    op0=ALU.mult, op1=ALU.mult,
                )
            nc.sync.dma_start(out=out_ap[bi, ti * P:(ti + 1) * P, :], in_=e)


_CACHE = {}


def _build(reps=1):
    if reps in _CACHE:
        return _CACHE[reps]
    nc = bacc.Bacc("TRN2", target_bir_lowering=False, debug=False,
                   num_devices=NCORES)
    m_ap = nc.dram_tensor("m", [BPC, N, N], F32, kind="ExternalInput").ap()
    out_ap = nc.dram_tensor("out", [BPC, N, N], F32, kind="ExternalOutput").ap()
    with tile.TileContext(nc) as tc:
        with ExitStack() as ctx:
            sinkhorn_kernel(ctx, tc, out_ap, m_ap, reps)
    nc.compile()
    _CACHE[reps] = nc
    return nc


def kernel(M: np.ndarray) -> np.ndarray:
    M = np.ascontiguousarray(M, dtype=np.float32)
    assert M.shape == (B, N, N)
    nc = _build()
    in_maps = [{"m": M[c * BPC:(c + 1) * BPC]} for c in range(NCORES)]
    res = run_bass_kernel_spmd(nc, in_maps, core_ids=list(range(NCORES)))
    return np.concatenate([res.results[c]["out"] for c in range(NCORES)], axis=0)


def _build_timing(loop_n):
    key = ("timing", loop_n)
    if key in _CACHE:
        return _CACHE[key]
    nc = bacc.Bacc("TRN2", target_bir_lowering=False, debug=False,
                   num_devices=NCORES)
    m_ap = nc.dram_tensor("m", [1, N, N], F32, kind="ExternalInput").ap()
    out_ap = nc.dram_tensor("out", [1, N, N], F32, kind="ExternalOutput").ap()
    with tile.TileContext(nc) as tc:
        with ExitStack() as ctx:
            with tc.For_i(0, loop_n, 1):
                sinkhorn_kernel(ctx, tc, out_ap, m_ap, reps=1, alias_io=True)
    nc.compile()
    _CACHE[key] = nc
    return nc


# revision 32
# speedup vs baseline: 1.1488x; 1.1488x over previous
"""Trainium2 Bass kernel for nn_SinkhornLayer: 10 log-domain Sinkhorn iterations
on 64 independent [1024,1024] fp32 matrices, batch-sharded over 8 NeuronCores.

Algorithm (multiplicative domain, mathematically equal to log-domain reference):
    K  = exp(10*(M - rowmax(M)))      rowmax for overflow safety
    u1 = 1 / rowsum(K)                (rowsum fused into the exp pass on ScalarE)
    for t = 1..10:
        if t > 1:  u = 1 / (K v)      row-direction matvec
        v = 1 / (K^T u)               col-direction matvec
    out = diag(u) K diag(v)

K and K^T are held in SBUF in bf16 (numpy simulation of the bf16 pipeline vs
the fp32 reference gives ~3.6e-3 max rel err, well under the 2e-2 gate).

Matvec form: the 128x128 K tiles are the stationary (weight) operand and the
vector chunk [128,1] is the moving operand, so every half-iteration is 64
LDWEIGHTS+MATMUL pairs accumulating into a [128,8] PSUM column vector -- the
u/v iterates stay in column layout the whole time and the only non-PE op per
half-iteration is one tiny reciprocal. bf16 weights get the fast-weight-load
path (2 cols/cycle).
"""
import numpy as np
from contextlib import ExitStack

import concourse.bacc as bacc
import concourse.bass as bass
import concourse.tile as tile
from concourse import mybir
from concourse.bass_utils import run_bass_kernel_spmd
from concourse.masks import make_identity

F32 = mybir.dt.float32
BF16 = mybir.dt.bfloat16
AF = mybir.ActivationFunctionType
ALU = mybir.AluOpType

P = 128          # SBUF partitions
N = 1024         # matrix dim
B = 64           # batch
NCORES = 8
BPC = B // NCORES
TPM = N // P     # 8 row/col tiles per matrix
ITERS = 10
INV_EPS = 10.0


COLTILE = False


# 2 concurrent PE column groups, each streaming its own j-slice of the matrix
# on its own XBUS (xdata[0]/xdata[1]; 4 groups hits the quadrant-3 HW bug and
# 3 groups proved unstable under For_i).
CT_SPLITS = ((0, 512), (512, 1024))
# (psum row, free offset) holding column chunk tj of the matvec result
CT_CHUNK = ((0, 0), (0, 128), (0, 256), (0, 384),
            (32, 0), (32, 128), (32, 256), (32, 384))


def _half_iter_ct(tc, pools, mat, w, ones_col, fp32_out=False):
    """Column-tiled streaming half-iteration: bf16 [P, TPM] = 1/(mat^T w).
    mat: [P, TPM, N] bf16, streamed as the moving operand through 3 concurrent
    PE column groups; w: [P, TPM] bf16 column vector (the stationary operand,
    M=1). Result rows {0,32,64} of PSUM are transposed back to column layout.
    The MM block is a scheduling-atomic critical section: a full-array PE op
    (e.g. a transpose) interleaved between col-tiled MMs with live concurrent
    group streams wedges the PE."""
    nc = tc.nc
    psmv, pscol, sbmv, sbvec = pools
    mv = psmv.tile([P, 512], F32, tag="mv")
    with tc.tile_critical():
        for ti in range(TPM):
            for g, (j0, j1) in enumerate(CT_SPLITS):
                nc.tensor.matmul(
                    mv[32 * g:32 * g + 1, 0:j1 - j0],
                    w[:, ti:ti + 1],
                    mat[:, ti, j0:j1],
                    start=(ti == 0), stop=(ti == TPM - 1),
                    tile_position=(0, 32 * g), skip_group_check=True,
                )
    mv_sb = sbmv.tile([P, 512], F32, tag="mv_sb")
    nc.vector.tensor_copy(mv_sb, mv)
    sc = pscol.tile([P, TPM], F32, tag="sc")
    for tj, (row, off) in enumerate(CT_CHUNK):
        nc.tensor.transpose(
            sc[:, tj:tj + 1],
            mv_sb[row:row + 1, off:off + P],
            ones_col[row:row + 1, 0:1],
        )
    rb = sbvec.tile([P, TPM], BF16, tag="uvb")
    with nc.allow_low_precision("sinkhorn u/v iterates are bf16 by design"):
        nc.vector.reciprocal(rb, sc)
    if fp32_out:
        rf = sbvec.tile([P, TPM], F32, tag="uvf")
        nc.vector.reciprocal(rf, sc)
        return rb, rf
    return rb, None


def _half_iter(nc, psmv, sbvec, mat, w, fp32_out=False):
    """One half-iteration: returns bf16 [P, TPM] = 1/(mat^T w) in column layout.
    mat: [P, TPM, N] bf16 tiles (contraction over partitions);
    w:   [P, TPM] bf16 column-layout vector.
    If fp32_out, also returns an fp32 copy of the reciprocal."""
    ps = psmv.tile([P, TPM], F32, tag="mv")
    for ob in range(TPM):
        for ci in range(TPM):
            nc.tensor.matmul(
                ps[:, ob:ob + 1],
                mat[:, ci, ob * P:(ob + 1) * P],
                w[:, ci:ci + 1],
                start=(ci == 0), stop=(ci == TPM - 1),
            )
    rb = sbvec.tile([P, TPM], BF16, tag="uvb")
    with nc.allow_low_precision("sinkhorn u/v iterates are bf16 by design"):
        nc.vector.reciprocal(rb, ps)
    if fp32_out:
        rf = sbvec.tile([P, TPM], F32, tag="uvf")
        nc.vector.reciprocal(rf, ps)
        return rb, rf
    return rb, None


def sinkhorn_kernel(ctx, tc, out_ap, m_ap, reps=1, alias_io=False):
    nc = tc.nc
    const = ctx.enter_context(tc.tile_pool(name="const", bufs=1))
    ident = const.tile([P, P], BF16)
    make_identity(nc, ident[:])
    identf = const.tile([P, P], F32)
    make_identity(nc, identf[:])
    ones_row = const.tile([1, P], F32)
    nc.vector.memset(ones_row, 1.0)
    ones_col = const.tile([P, 1], F32)
    nc.vector.memset(ones_col, 1.0)

    kpool = ctx.enter_context(tc.tile_pool(name="kmat", bufs=2))
    ktpool = ctx.enter_context(tc.tile_pool(name="ktmat", bufs=2))
    ppool = ctx.enter_context(tc.tile_pool(name="p0", bufs=3))
    epool = ctx.enter_context(tc.tile_pool(name="eout", bufs=3))
    sbvec = ctx.enter_context(tc.tile_pool(name="sbvec", bufs=4))
    sbrow = ctx.enter_context(tc.tile_pool(name="sbrow", bufs=2))

    psmv = ctx.enter_context(tc.tile_pool(name="psmv", bufs=2, space="PSUM"))
    pscol = ctx.enter_context(tc.tile_pool(name="pscol", bufs=2, space="PSUM"))
    pstr = ctx.enter_context(tc.tile_pool(name="pstr", bufs=2, space="PSUM"))
    psbig = ctx.enter_context(tc.tile_pool(name="psbig", bufs=2, space="PSUM"))
    sbmv = ctx.enter_context(tc.tile_pool(name="sbmv", bufs=2))
    mv_pools = (psmv, pscol, sbmv, sbvec)

    for rep in range(reps):
      for b in range(BPC):
        bi = 0 if alias_io else b
        # ---- phase 1: load, rowmax, K = exp(10*(M - rowmax)) bf16, rowsum ----
        ktb = kpool.tile([P, TPM, N], BF16, tag="ktb")
        negmx = sbvec.tile([P, TPM], F32, tag="negmx")
        rowsum = sbvec.tile([P, TPM], F32, tag="rowsum")
        for ti in range(TPM):
            p0 = ppool.tile([P, N], F32, tag="p0")
            nc.sync.dma_start(out=p0, in_=m_ap[bi, ti * P:(ti + 1) * P, :])
            nc.vector.reduce_max(negmx[:, ti:ti + 1], p0,
                                 axis=mybir.AxisListType.X, negate=True)
            nc.vector.tensor_scalar_mul(negmx[:, ti:ti + 1], negmx[:, ti:ti + 1],
                                        INV_EPS)
            nc.scalar.activation(out=ktb[:, ti, :], in_=p0, func=AF.Exp,
                                 bias=negmx[:, ti:ti + 1], scale=INV_EPS,
                                 accum_out=rowsum[:, ti:ti + 1])
        ub = sbvec.tile([P, TPM], BF16, tag="uvb")
        with nc.allow_low_precision("sinkhorn u/v iterates are bf16 by design"):
            nc.vector.reciprocal(ub, rowsum)

        # ---- phase 2: K^T (bf16) via 64 PE block transposes, 8 PSUM drains ----
        kttb = ktpool.tile([P, TPM, N], BF16, tag="kttb")
        for tj in range(TPM):
            pt = pstr.tile([P, N], BF16, tag="pt")
            for ti in range(TPM):
                nc.tensor.transpose(pt[:, ti * P:(ti + 1) * P],
                                    ktb[:, ti, tj * P:(tj + 1) * P], ident)
            nc.vector.tensor_copy(kttb[:, tj, :], pt)

        # ---- phase 3: Sinkhorn iterations (bf16 weights, column vectors) ----
        u, v = ub, None
        uf = vf = None
        for t in range(ITERS):
            last = (t == ITERS - 1)
            if COLTILE:
                if t > 0:
                    u, uf = _half_iter_ct(tc, mv_pools, kttb, v, ones_col,
                                          fp32_out=last)
                v, vf = _half_iter_ct(tc, mv_pools, ktb, u, ones_col,
                                      fp32_out=last)
            else:
                if t > 0:
                    u, uf = _half_iter(nc, psmv, sbvec, kttb, v, fp32_out=last)
                v, vf = _half_iter(nc, psmv, sbvec, ktb, u, fp32_out=last)

        # ---- phase 4: out = diag(u) K diag(v) ----
        # v (fp32, col layout) -> row [1, N] via 8 PE transposes, broadcast to
        # [P, N] with a ones-matmul, then one fused DVE op per row chunk.
        vrow_sb = sbrow.tile([1, N], F32, tag="vrow")
        for h in range(2):
            vr_ps = pscol.tile([1, N // 2], F32, tag="sc")
            for k in range(4):
                tj = 4 * h + k
                nc.tensor.transpose(vr_ps[0:1, k * P:(k + 1) * P],
                                    vf[:, tj:tj + 1], identf)
            nc.any.tensor_copy(vrow_sb[0:1, h * (N // 2):(h + 1) * (N // 2)], vr_ps)
        vb = []
        for h in range(2):
            vbh = psbig.tile([P, N // 2], F32, tag="psb")
            nc.tensor.matmul(vbh, ones_row,
                             vrow_sb[0:1, h * (N // 2):(h + 1) * (N // 2)],
                             start=True, stop=True)
            vb.append(vbh)
        for ti in range(TPM):
            e = epool.tile([P, N], F32, tag="e")
            for h in range(2):
                nc.vector.scalar_tensor_tensor(
                    out=e[:, h * (N // 2):(h + 1) * (N // 2)],
                    in0=ktb[:, ti, h * (N // 2):(h + 1) * (N // 2)],
                    scalar=uf[:, ti:ti + 1],
                    in1=vb[h],
                    op0=ALU.mult, op1=ALU.mult,
                )
            nc.sync.dma_start(out=out_ap[bi, ti * P:(ti + 1) * P, :], in_=e)


_CACHE = {}


def _build(reps=1):
    if reps in _CACHE:
        return _CACHE[reps]
    nc = bacc.Bacc("TRN2", target_bir_lowering=False, debug=False,
                   num_devices=NCORES)
    m_ap = nc.dram_tensor("m", [BPC, N, N], F32, kind="ExternalInput").ap()
    out_ap = nc.dram_tensor("out", [BPC, N, N], F32, kind="ExternalOutput").ap()
    with tile.TileContext(nc) as tc:
        with ExitStack() as ctx:
            sinkhorn_kernel(ctx, tc, out_ap, m_ap, reps)
    nc.compile()
    _CACHE[reps] = nc
    return nc


def kernel(M: np.ndarray) -> np.ndarray:
    M = np.ascontiguousarray(M, dtype=np.float32)
    assert M.shape == (B, N, N)
    nc = _build()
    in_maps = [{"m": M[c * BPC:(c + 1) * BPC]} for c in range(NCORES)]
    res = run_bass_kernel_spmd(nc, in_maps, core_ids=list(range(NCORES)))
    return np.concatenate([res.results[c]["out"] for c in range(NCORES)], axis=0)


def _build_timing(loop_n):
    key = ("timing", loop_n)
    if key in _CACHE:
        return _CACHE[key]
    nc = bacc.Bacc("TRN2", target_bir_lowering=False, debug=False,
                   num_devices=NCORES)
    m_ap = nc.dram_tensor("m", [1, N, N], F32, kind="ExternalInput").ap()
    out_ap = nc.dram_tensor("out", [1, N, N], F32, kind="ExternalOutput").ap()
    with tile.TileContext(nc) as tc:
        with ExitStack() as ctx:
            with tc.For_i(0, loop_n, 1):
                sinkhorn_kernel(ctx, tc, out_ap, m_ap, reps=1, alias_io=True)
    nc.compile()
    _CACHE[key] = nc
    return nc
